# revision 1
# baseline (speedup 1.0000x reference)
"""Trainium2 Bass kernel for nn_Conv2d_68298569941797.

Conv2d: data [32,1,224,224] f32 (x) weight [64,1,3,3] f32 -> out [32,64,222,222] f32
(valid padding, stride 1, cross-correlation).

Strategy (data-parallel over batch, 4 images per NeuronCore x 8 cores):
  The conv is lowered to a single stationary matmul per output chunk.
  Output rows are split into two halves (0..110 / 111..221). The stationary
  operand lhsT is [K=18, M=128]: K = (half, ky, kx), M = (half, out_channel),
  with zeros in the cross-half blocks. The moving operand rhs [18, N] is read
  from 18 shifted copies of the image resident in SBUF: partition
  k = (h, ky, kx) holds the image shifted by (111*h + ky) rows and kx cols.
  One matmul column computes all 128 = 2x64 outputs for one output pixel pair
  ((y, x) for half 0 and (y+111, x) for half 1).

  Chunks: 2 output rows x 222 cols = 444 columns per matmul (fits one PSUM
  bank, and N>=256 keeps float32r matmul at 1 cycle/row). 4 chunks stage into
  one SBUF tile [128, 1776] whose free dim maps to 8 contiguous output rows,
  so the output DMA writes contiguous 7104B runs per (half, channel).

This file is self-contained: shapes/sharding are hardcoded; it only imports
installed packages (numpy, concourse).
"""

import numpy as np

import concourse.bass as bass
import concourse.mybir as mybir
import concourse.tile as tile
from concourse import bacc
from concourse.bass_utils import run_bass_kernel_spmd

N_CORES = 8
B, H, W = 32, 224, 224
O, KH, KW = 64, 3, 3
OH, OW = 222, 222
BPC = B // N_CORES          # images per core
HALF = OH // 2              # 111 output rows per half
KP = 18                     # contraction: (half, ky, kx)
M = 128                     # outputs per column: (half, out_channel)
SEG = 111 * W               # 24864: contiguous elems loaded per partition
IMG = H * W                 # 50176
DATA_LEN = BPC * IMG + 2    # flat padded per-core input (+2: shift-window tail)
OIMG = O * OH * OW          # per-image output elems
CHUNK_ROWS = 2              # output rows per matmul chunk
CHUNK_N = CHUNK_ROWS * OW   # 444 matmul columns
BLK_CHUNKS = 8              # chunks per staged output DMA
BLK_N = BLK_CHUNKS * CHUNK_N
# block base rows: 6 blocks of 16 rows + one final overlapping block
BLOCK_YS = [16 * j for j in range(6)] + [95]

MM_DT = mybir.dt.float32r

# bigblk: 4 output blocks per image (16/16/16/8 chunks = 32/32/32/16 rows,
# last starts at 95 so row 95 is written twice with identical data) instead
# of 7x8 chunks — halves the DMA count, doubles the contiguous run to 28KB.
BIGBLOCKS = [(0, 16), (32, 16), (64, 16), (95, 8)]
_VIDX = {"full": 0, "bigblk": 1, "dmaonly": 2, "noout": 3}


def nonce_len(reps, variant):
    # Distinct per-configuration input shape: the axon PJRT executable cache
    # keys on HLO structure only (the BIR rides in an opaque backend_config),
    # so rep-count/variant twins would otherwise collide on one cached NEFF.
    return 16 * (1 + reps) + 8 * _VIDX.get(variant, 4)


def _build_body(tc, data_ap, weight_ap, out_ap, nonce_ap,
                reps=1, variant="full"):
    nc = tc.nc
    data_t = data_ap.tensor
    weight_t = weight_ap.tensor
    out_t = out_ap.tensor
    nonce_t = nonce_ap.tensor
    nlen = nonce_len(reps, variant)
    do_in = variant not in ("noin",)
    do_mm = variant not in ("nocompute", "dmaonly")
    do_out = variant not in ("noout",)
    blocks = (BIGBLOCKS if variant in ("full", "bigblk")
              else [(Y, BLK_CHUNKS) for Y in BLOCK_YS])
    maxn = max(n for _, n in blocks) * CHUNK_N
    # measured A/B (same-session reps-17 differential): v1 single-ring
    # structure 1306us, +bigblocks alone 1310us, v2 (bigblocks + outputs
    # alternating sync/scalar HWDGE rings + inputs on SWDGE) 1119us.
    dual_ring = variant in ("full", "bigblk")

    with (
        tc.tile_pool(name="const", bufs=1) as const_pool,
        tc.tile_pool(name="imgp", bufs=1) as img_pool,
        tc.tile_pool(name="psp", bufs=8, space="PSUM") as psum_pool,
        tc.tile_pool(name="stp", bufs=3) as stage_pool,
    ):
        # lhsT [18, 128]: host-prescattered (see make_in_maps), loaded with a
        # single SWDGE DMA that casts f32 -> f32r (the fast fp32 matmul
        # format; producers of f32r-consumed data must write f32r).
        lhsT = const_pool.tile([KP, M], MM_DT)
        nc.sync.dma_start(lhsT[:], bass.AP(weight_t, 0, [[M, KP], [1, M]]))
        # one tiny load outside the reps loop: constant cost, cancels in the
        # differential; exists only to make this configuration's HLO unique
        nt = const_pool.tile([1, nlen], mybir.dt.float32)
        nc.sync.dma_start(nt[:], bass.AP(nonce_t, 0, [[nlen, 1], [1, nlen]]))

        for b in [b for _ in range(reps) for b in range(BPC)]:
            # 18 shifted image copies; partition k=(h,ky,kx) holds the
            # contiguous window data[b].flat[(111h+ky)*224+kx :][:SEG]
            img3 = img_pool.tile([KP, 111, W], MM_DT)
            if do_in:
                # 4 loads: [9 partitions, band] each; the 9 shifted copies
                # come from (ky, kx) source dims. On the SWDGE (gpsimd)
                # path so they never queue behind output DMAs on the two
                # HWDGE rings.
                in_eng = nc.gpsimd if dual_ring else nc.scalar
                for h in range(2):
                    for r0, R in ((0, 56), (56, 55)):
                        src = bass.AP(
                            data_t, b * IMG + (HALF * h + r0) * W,
                            [[W, 3], [1, 3], [1, R * W]],
                        )
                        in_eng.dma_start(
                            img3[h * 9:(h + 1) * 9, r0:r0 + R, :], src)

            for bi, (Y, nch) in enumerate(blocks):
                blk_n = nch * CHUNK_N
                stage = stage_pool.tile([M, maxn], mybir.dt.float32)
                if variant == "dmaonly":
                    nc.gpsimd.memset(stage[:], 0)
                if do_mm:
                    for i in range(nch):
                        y0 = Y + CHUNK_ROWS * i
                        ps = psum_pool.tile([M, CHUNK_N], mybir.dt.float32)
                        rhs = img3[:, y0:y0 + CHUNK_ROWS, 0:OW]
                        nc.tensor.matmul(
                            ps[:], lhsT[:], rhs,
                            start=True, stop=True,
                        )
                        nc.vector.tensor_copy(
                            stage[:, i * CHUNK_N:(i + 1) * CHUNK_N], ps[:])
                if do_out:
                    # stage free dim = contiguous output rows starting at Y
                    dest = bass.AP(
                        out_t, b * OIMG + Y * OW,
                        [[HALF * OW, 2], [OH * OW, 64], [1, blk_n]],
                    )
                    eng = (nc.scalar if dual_ring and bi % 2 == 1
                           else nc.sync)
                    eng.dma_start(dest, stage[:, 0:blk_n])


_NC_CACHE = {}


def _get_nc(reps=1, variant="full"):
    key = (reps, variant)
    if key not in _NC_CACHE:
        nc = bacc.Bacc(
            "TRN2",
            target_bir_lowering=False,
            debug=False,
            num_devices=N_CORES,
        )
        data = nc.dram_tensor(
            "data", [DATA_LEN], MM_DT, kind="ExternalInput").ap()
        weight = nc.dram_tensor(
            "weight", [KP, M], MM_DT,
            kind="ExternalInput").ap()
        out = nc.dram_tensor(
            "out", [BPC, O, OH, OW], mybir.dt.float32,
            kind="ExternalOutput").ap()
        nonce = nc.dram_tensor(
            "nonce", [nonce_len(reps, variant)], mybir.dt.float32,
            kind="ExternalInput").ap()
        with tile.TileContext(nc) as tc:
            _build_body(tc, data, weight, out, nonce,
                        reps=reps, variant=variant)
        nc.compile()
        _NC_CACHE[key] = nc
    return _NC_CACHE[key]


def make_in_maps(data, weight):
    data = np.ascontiguousarray(np.asarray(data), dtype=np.float32)
    weight = np.ascontiguousarray(np.asarray(weight), dtype=np.float32)
    # host-side scatter of w[o,0,ky,kx] into lhsT [K=(h,ky,kx), M=(h,o)]
    lhsT = np.zeros((KP, M), np.float32)
    blk = weight[:, 0].transpose(1, 2, 0).reshape(9, O)  # [(ky,kx), o]
    for h in range(2):
        lhsT[h * 9:(h + 1) * 9, h * O:(h + 1) * O] = blk
    in_maps = []
    for c in range(N_CORES):
        flat = data[c * BPC:(c + 1) * BPC].reshape(-1)
        flat = np.concatenate([flat, np.zeros(2, np.float32)])
        in_maps.append({"data": flat, "weight": lhsT})
    return in_maps


def kernel(data, weight):
    nc = _get_nc()
    in_maps = make_in_maps(data, weight)
    nz = np.zeros(nonce_len(1, "full"), np.float32)
    for m in in_maps:
        m["nonce"] = nz
    res = run_bass_kernel_spmd(
        nc, in_maps, core_ids=list(range(N_CORES)))
    return np.concatenate([r["out"] for r in res.results], axis=0)

